# revision 5
# baseline (speedup 1.0000x reference)
"""GATv2 (2 layers, H=4, C=128, head-mean) on 8 TRN2 cores, dst-partitioned.

v2 design (per layer, one SPMD launch of a shared single-layer NEFF):
  dense: xl'' = x @ (Wl .* |att|) for ALL nodes (bf16, to DRAM);
         xr'' per local block -> fp8 into slot 0 of the fp8 gather tile.
  per 128-edge chunk (edges sorted by dst, 10 blocks x 128 dst/core):
    - SWDGE row-gather xl''[src] (bf16) into slot k of the g tile
    - bf16->fp8 conversion of the gathered slot (round-robin Pool/DVE/ACT)
    - transposed u: per head one fp8 DoubleRow matmul computes
      uT[c,e] = sum_d xr[d,c]*oht[d,e] + g[e,c]  (both contractions packed
      as the two DoubleRow halves via slot-strided APs)
    - prelu on ACT -> s'T bf16
    - logits: per head a [128,1] matmul  z[e] = sum_c s'T[c,e]*sgn[c]
    - ez = e^z via DVE pow (f32), per chunk-pair
    - eg = g .* ez (4x tensor_scalar per chunk, bf16)
    - agg += oh^T @ eg, den += oh^T @ ez (PSUM accumulation over chunks)
  tail per block: alpha = agg*rden, .*(0.25/|att|), head-sum, +residual,
  LayerNorm (rstd = exp(-0.5*ln(var+eps))), relu.
Host: edge sorting, fp8 one-hot (oht|id) and oh arrays, wrapped gather idxs.
"""

from contextlib import ExitStack

import numpy as np
import ml_dtypes

import concourse.bacc as bacc
import concourse.tile as tile
from concourse import mybir
from concourse.bass_utils import run_bass_kernel_spmd

BF16 = ml_dtypes.bfloat16
FP8 = ml_dtypes.float8_e4m3fn

N_NODES = 10000
D = 128
H = 4
C = 128
HC = H * C
NEG_SLOPE = 0.2
LN_EPS = 1e-5
L = 2

N_CORES = 8
NODES_PER_CORE = 1280
BLOCKS = 10
BLK = 128
N_PAD = N_CORES * NODES_PER_CORE    # 10240
N_ROWS = 10112                      # 79*128
N_TILES = N_ROWS // 128

_NC_CACHE = {}
LAST_RESULTS = []   # BassKernelResults per launch (for test harness)

# conversion engine per chunk index: mostly Pool, some DVE, a little ACT
_CONV_PAT = ["ACT", "DVE", "POOL", "POOL", "DVE", "POOL",
             "POOL", "DVE", "POOL", "POOL", "DVE"]


def _prep_edges(edge_index):
    src = np.concatenate([np.asarray(edge_index[0], np.int64),
                          np.arange(N_NODES, dtype=np.int64)])
    dst = np.concatenate([np.asarray(edge_index[1], np.int64),
                          np.arange(N_NODES, dtype=np.int64)])
    pad_nodes = np.arange(N_NODES, N_PAD, dtype=np.int64)
    src = np.concatenate([src, np.zeros_like(pad_nodes)])
    dst = np.concatenate([dst, pad_nodes])

    order = np.argsort(dst, kind="stable")
    src = src[order]
    dst = dst[order]

    blk_of_edge = dst // BLK
    n_blocks_total = N_PAD // BLK
    counts = np.bincount(blk_of_edge, minlength=n_blocks_total)
    K = int(np.max((counts + BLK - 1) // BLK))
    K += K % 2  # even, so we can process chunk pairs

    cap = K * BLK
    src_arr = np.zeros((n_blocks_total, cap), np.int32)
    dpos_arr = np.full((n_blocks_total, cap), -1, np.int32)
    block_starts = np.zeros(n_blocks_total + 1, np.int64)
    np.cumsum(counts, out=block_starts[1:])
    slot = np.arange(len(dst)) - block_starts[blk_of_edge]
    src_arr[blk_of_edge, slot] = src.astype(np.int32)
    dpos_arr[blk_of_edge, slot] = (dst - blk_of_edge * BLK).astype(np.int32)

    return (K, src_arr.reshape(N_CORES, BLOCKS, cap),
            dpos_arr.reshape(N_CORES, BLOCKS, cap))


def _build_ship_arrays(K, src_arr, dpos_arr):
    cap = K * BLK
    # wrapped gather indices: idx i lives at [i % 16, i // 16]; the 16-row
    # pattern is tiled 8x along partitions (one copy per SWDGE Q7 core).
    s = src_arr.reshape(N_CORES, BLOCKS, cap // 16, 16)
    s = np.swapaxes(s, 2, 3)                                  # [c,b,16,cap/16]
    sidx = np.tile(s, (1, 1, 8, 1)).astype(np.int16)          # [c,b,128,cap/16]

    # fp8 one-hots:
    # ohtid [c,b, d(128), (K+1)*128]: slot k col k*128+e -> oht[d,e]=1 iff
    #   dst(chunk k, e) == d; slot K = identity (rows e).
    # ohflat [c,b, e(128), cap]: col k*128+d -> oh[e, k, d]
    ohtid = np.zeros((N_CORES, BLOCKS, BLK, (K + 1) * BLK), FP8)
    ohflat = np.zeros((N_CORES, BLOCKS, BLK, cap), FP8)
    cc, bb, ss = np.nonzero(dpos_arr >= 0)
    kk = (ss // BLK).astype(np.int64)
    ee = (ss % BLK).astype(np.int64)
    dd = dpos_arr[cc, bb, ss].astype(np.int64)
    ohtid[cc, bb, dd, kk * BLK + ee] = 1
    ohflat[cc, bb, ee, kk * BLK + dd] = 1
    i = np.arange(BLK)
    ohtid[:, :, i, K * BLK + i] = 1
    return (np.ascontiguousarray(sidx), np.ascontiguousarray(ohtid),
            np.ascontiguousarray(ohflat))


def _bcast(v, rows=128):
    v = np.asarray(v, np.float32)
    return np.ascontiguousarray(np.broadcast_to(v[None, :], (rows, v.shape[0])))


def _build_nc(K, bias_zero, ln_triv):
    nc = bacc.Bacc("TRN2", target_bir_lowering=False, debug=False,
                   num_devices=N_CORES)
    f32, bf16, i16 = mybir.dt.float32, mybir.dt.bfloat16, mybir.dt.int16
    fp8 = mybir.dt.float8e4
    AF = mybir.ActivationFunctionType
    ALU = mybir.AluOpType
    PM = mybir.MatmulPerfMode
    X = mybir.AxisListType.X
    cap = K * BLK

    xT = nc.dram_tensor("xT", [128, N_ROWS], bf16, kind="ExternalInput")
    xlocT = nc.dram_tensor("xlocT", [128, NODES_PER_CORE], bf16,
                           kind="ExternalInput")
    xloc = nc.dram_tensor("xloc", [NODES_PER_CORE, 128], f32,
                          kind="ExternalInput")
    WlS = nc.dram_tensor("WlS", [128, HC], bf16, kind="ExternalInput")
    WrS = nc.dram_tensor("WrS", [128, HC], bf16, kind="ExternalInput")
    blB = nc.dram_tensor("blB", [128, HC], f32, kind="ExternalInput")
    brB = nc.dram_tensor("brB", [128, HC], f32, kind="ExternalInput")
    sgnT = nc.dram_tensor("sgnT", [128, H], bf16, kind="ExternalInput")
    invatt4B = nc.dram_tensor("invatt4B", [128, HC], f32, kind="ExternalInput")
    biasB = nc.dram_tensor("biasB", [128, 128], f32, kind="ExternalInput")
    lngB = nc.dram_tensor("lngB", [128, 128], f32, kind="ExternalInput")
    lnbB = nc.dram_tensor("lnbB", [128, 128], f32, kind="ExternalInput")
    ohtidd = nc.dram_tensor("ohtidd", [BLOCKS, BLK, (K + 1) * BLK], fp8,
                            kind="ExternalInput")
    ohd = nc.dram_tensor("ohd", [BLOCKS, BLK, cap], fp8, kind="ExternalInput")
    sidxd = nc.dram_tensor("sidxd", [BLOCKS, 128, cap // 16], i16,
                           kind="ExternalInput")

    xnew = nc.dram_tensor("xnew", [NODES_PER_CORE, 128], f32,
                          kind="ExternalOutput")

    with tile.TileContext(nc) as tc, ExitStack() as ctx:
        consts = ctx.enter_context(tc.tile_pool(name="consts", bufs=1))
        lhsp = ctx.enter_context(tc.tile_pool(name="lhs", bufs=3))
        densep = ctx.enter_context(tc.tile_pool(name="dense", bufs=3))
        gp = ctx.enter_context(tc.tile_pool(name="g", bufs=2))
        g8p = ctx.enter_context(tc.tile_pool(name="g8", bufs=2))
        otp = ctx.enter_context(tc.tile_pool(name="ot", bufs=2))
        ohp = ctx.enter_context(tc.tile_pool(name="ohf", bufs=2))
        sxp = ctx.enter_context(tc.tile_pool(name="sx", bufs=2))
        sp = ctx.enter_context(tc.tile_pool(name="s", bufs=3))
        ezp = ctx.enter_context(tc.tile_pool(name="ez", bufs=4))
        egp = ctx.enter_context(tc.tile_pool(name="eg", bufs=3))
        lnp = ctx.enter_context(tc.tile_pool(name="ln", bufs=2))
        lgp = ctx.enter_context(tc.tile_pool(name="lg", bufs=4))
        outp = ctx.enter_context(tc.tile_pool(name="out", bufs=2))
        dramp = ctx.enter_context(tc.tile_pool(name="dram", bufs=1,
                                               space="DRAM"))
        pup = ctx.enter_context(tc.tile_pool(name="pu", bufs=2, space="PSUM"))
        pzp = ctx.enter_context(tc.tile_pool(name="pz", bufs=1, space="PSUM"))
        pdenp = ctx.enter_context(tc.tile_pool(name="pden", bufs=1,
                                               space="PSUM"))
        paggp = ctx.enter_context(tc.tile_pool(name="pagg", bufs=2,
                                               space="PSUM"))

        def load_const(src_ap, shape, dtype, name):
            t = consts.tile(shape, dtype, tag=name)
            nc.sync.dma_start(t[:], src_ap)
            return t

        wl_sb = load_const(WlS[:], [128, HC], bf16, "wl")
        wr_sb = load_const(WrS[:], [128, HC], bf16, "wr")
        sgn_sb = load_const(sgnT[:], [128, H], bf16, "sgn")
        invatt_sb = load_const(invatt4B[:], [128, HC], f32, "invatt")
        if not bias_zero:
            blB_sb = load_const(blB[:], [128, HC], f32, "blB")
            brB_sb = load_const(brB[:], [128, HC], f32, "brB")
            biasB_sb = load_const(biasB[:], [128, 128], f32, "biasB")
        if not ln_triv:
            lngB_sb = load_const(lngB[:], [128, 128], f32, "lngB")
            lnbB_sb = load_const(lnbB[:], [128, 128], f32, "lnbB")

        xl_dram = dramp.tile([N_ROWS, HC], bf16)

        alphaP = consts.tile([128, 1], f32, tag="alphaP")
        nc.vector.memset(alphaP[:], NEG_SLOPE)

        blk_loads = {}
        blk_g = {}

        def prefetch_loads(b):
            six = sxp.tile([128, cap // 16], i16, tag="sidx")
            nc.sync.dma_start(six[:], sidxd[b])
            ot = otp.tile([128, K + 1, BLK], fp8, tag="ot")
            nc.sync.dma_start(
                ot[:], ohtidd[b].rearrange("p (k e) -> p k e", e=BLK))
            ohb = ohp.tile([128, cap], fp8, tag="oh")
            nc.sync.dma_start(ohb[:], ohd[b])
            blk_loads[b] = (six, ot, ohb)

        def prefetch_gather(b, splits=3):
            six, ot, ohb = blk_loads[b]
            g = gp.tile([128, K + 1, HC], bf16, tag="g")
            bounds = [K * i // splits for i in range(splits + 1)]
            for i in range(splits):
                k0, k1 = bounds[i], bounds[i + 1]
                n_idx = (k1 - k0) * BLK
                nc.gpsimd.dma_gather(
                    out_ap=g[:, 1 + k0:1 + k1, :], in_ap=xl_dram[:],
                    idxs_ap=six[:, k0 * BLK // 16:k1 * BLK // 16],
                    num_idxs=n_idx, num_idxs_reg=n_idx, elem_size=HC,
                    single_packet=False)
            blk_g[b] = g

        # ---- dense: xl'' for all nodes -> DRAM bf16 ----
        # single batched xT load; xl_dram stores batched 4 tiles at a time
        xT_sb = consts.tile([128, N_ROWS], bf16, tag="xT")
        nc.sync.dma_start(xT_sb[:], xT[:])
        prefetch_loads(0)
        GB = 4
        for t0 in range(0, N_TILES, GB):
            n_sub = min(GB, N_TILES - t0)
            xs4 = densep.tile([128, GB, HC], bf16, tag="xs4")
            for j in range(n_sub):
                t_i = t0 + j
                if j % 2 == 0:
                    ps = pup.tile([128, 2, HC], f32, tag="uT")
                nc.tensor.matmul(ps[:, j % 2, :],
                                 xT_sb[:, t_i * 128:(t_i + 1) * 128],
                                 wl_sb[:], start=True, stop=True)
                if bias_zero:
                    if t_i % 2 == 0:
                        nc.scalar.activation(xs4[:, j, :], ps[:, j % 2, :],
                                             AF.Copy)
                    else:
                        nc.vector.tensor_scalar(out=xs4[:, j, :],
                                                in0=ps[:, j % 2, :],
                                                scalar1=1.0, scalar2=None,
                                                op0=ALU.mult)
                else:
                    nc.vector.tensor_tensor(out=xs4[:, j, :],
                                            in0=ps[:, j % 2, :],
                                            in1=blB_sb[:], op=ALU.add)
            nc.sync.dma_start(
                xl_dram[t0 * 128:(t0 + n_sub) * 128, :].rearrange(
                    "(t p) c -> p t c", p=128),
                xs4[:, :n_sub, :])

        # ---- edge: flattened cross-block software pipeline ----
        # Global pair list; stage offsets (conv @i, u+prelu @i-1, z/exp/eg
        # @i-2, agg/den @i-3) run across block boundaries so no engine
        # queue drains at block edges. Block tails are emitted right after
        # the block's last agg/den.
        prefetch_gather(0, splits=6)
        P = K // 2
        pairs = [(b, p) for b in range(BLOCKS) for p in range(P)]
        NPAIR = len(pairs)
        st = {"g8": {}, "agg": {}, "den": {}, "s": {}, "ez": {}, "eg": {}}

        def blk_state(b):
            if b not in st["g8"]:
                g8 = g8p.tile([128, K + 1, HC], fp8, tag="g8")
                st["g8"][b] = g8
                # xr'' for this block -> fp8 slot 0
                lhs = lhsp.tile([128, 128], bf16, tag="lhs")
                nc.sync.dma_start(lhs[:], xlocT[:, b * 128:(b + 1) * 128])
                psr2 = pup.tile([128, 2, HC], f32, tag="uT")
                psr = psr2[:, 0, :]
                nc.tensor.matmul(psr, lhs[:], wr_sb[:], start=True, stop=True)
                if bias_zero:
                    nc.scalar.activation(g8[:, 0, :], psr, AF.Copy)
                else:
                    xrt = densep.tile([128, HC], bf16, tag="xrt")
                    nc.vector.tensor_tensor(out=xrt[:], in0=psr,
                                            in1=brB_sb[:], op=ALU.add)
                    nc.scalar.activation(g8[:, 0, :], xrt[:], AF.Copy)
                agg_t = paggp.tile([128, HC], f32, tag="agg")
                den_t = pdenp.tile([128, 4], f32, tag="den")
                st["agg"][b] = agg_t
                st["den"][b] = den_t
            return st["g8"][b], st["agg"][b], st["den"][b]

        def conv_stage(b, p):
            g8, _, _ = blk_state(b)
            g = blk_g[b]
            for j in (0, 1):
                slot = 2 * p + j + 1
                nc.vector.tensor_scalar(
                    out=g8[:, slot, :], in0=g[:, slot, :],
                    scalar1=1.0, scalar2=None, op0=ALU.mult)
            if p == 15 and b + 1 < BLOCKS:
                prefetch_loads(b + 1)
            if p == 16 and b + 1 < BLOCKS:
                prefetch_gather(b + 1)

        def front(b, p):
            g8, _, _ = blk_state(b)
            ot = blk_loads[b][1]
            uT = pup.tile([128, 2, HC], f32, tag="uT")
            for j in (0, 1):
                k = 2 * p + j
                slot = k + 1
                for h in range(H):
                    lhs_ap = g8[:, 0:slot + 1:slot, h * 128:(h + 1) * 128]
                    rhs_ap = ot[:, k:K + 1:K - k, :]
                    nc.tensor.matmul(uT[:, j, h * 128:(h + 1) * 128],
                                     lhs_ap, rhs_ap, start=True, stop=True,
                                     perf_mode=PM.DoubleRow)
            s_ = sp.tile([128, 2, HC], bf16, tag="s")
            nc.scalar.activation(s_[:], uT[:], AF.Prelu, alpha=alphaP[:])
            st["s"][(b, p)] = s_

        def mid(b, p):
            g = blk_g[b]
            zP = pzp.tile([128, 8], f32, tag="zP")
            s_ = st["s"].pop((b, p))
            for j in (0, 1):
                for h in range(H):
                    nc.tensor.matmul(zP[:, j * 4 + h:j * 4 + h + 1],
                                     s_[:, j, h * 128:(h + 1) * 128],
                                     sgn_sb[:, h:h + 1],
                                     start=True, stop=True)
            ezf = ezp.tile([128, 8], f32, tag="ez")
            nc.scalar.activation(ezf[:], zP[:], AF.Exp)
            ezb = ezp.tile([128, 8], bf16, tag="ezb")
            nc.vector.tensor_scalar(out=ezb[:], in0=ezf[:], scalar1=1.0,
                                    scalar2=None, op0=ALU.mult)
            st["ez"][(b, p)] = ezb
            eg = egp.tile([128, 2, HC], bf16, tag="eg")
            for j in (0, 1):
                k = 2 * p + j
                for h in range(H):
                    nc.gpsimd.tensor_scalar(
                        out=eg[:, j, h * 128:(h + 1) * 128],
                        in0=g[:, k + 1, h * 128:(h + 1) * 128],
                        scalar1=ezf[:, j * 4 + h:j * 4 + h + 1],
                        scalar2=None, op0=ALU.mult)
            st["eg"][(b, p)] = eg

        def back(b, p):
            _, agg, den = blk_state(b)
            ohb = blk_loads[b][2]
            eg = st["eg"].pop((b, p))
            ezb = st["ez"].pop((b, p))
            for j in (0, 1):
                k = 2 * p + j
                nc.tensor.matmul(agg[:], ohb[:, k * BLK:(k + 1) * BLK],
                                 eg[:, j, :],
                                 start=(k == 0), stop=(k == K - 1))
                nc.tensor.matmul(den[:], ohb[:, k * BLK:(k + 1) * BLK],
                                 ezb[:, j * 4:(j + 1) * 4],
                                 start=(k == 0), stop=(k == K - 1))
            if p == P - 1:
                tail(b)

        def tail(b):
            agg = st["agg"].pop(b)
            den = st["den"].pop(b)
            for dd in ("g8",):
                st[dd].pop(b, None)
            blk_g.pop(b, None)
            blk_loads.pop(b, None)
            rden = lgp.tile([128, 4], f32, tag="rden")
            nc.vector.reciprocal(rden[:], den[:])
            tq = lnp.tile([128, HC], f32, tag="tq")
            for h in range(H):
                nc.vector.tensor_scalar(
                    out=tq[:, h * 128:(h + 1) * 128],
                    in0=agg[:, h * 128:(h + 1) * 128],
                    scalar1=rden[:, h:h + 1], scalar2=None, op0=ALU.mult)
            tq2 = lnp.tile([128, HC], f32, tag="tq2")
            nc.vector.tensor_tensor(out=tq2[:], in0=tq[:], in1=invatt_sb[:],
                                    op=ALU.mult)
            hm = outp.tile([128, 128], f32, tag="hm")
            nc.vector.tensor_reduce(
                out=hm[:], in_=tq2[:].rearrange("p (h c) -> p c h", h=H),
                axis=X, op=ALU.add)
            xt = outp.tile([128, 128], f32, tag="xres")
            nc.sync.dma_start(xt[:], xloc[b * 128:(b + 1) * 128, :])
            if bias_zero:
                r2 = outp.tile([128, 128], f32, tag="r2")
                nc.vector.tensor_tensor(out=r2[:], in0=hm[:], in1=xt[:],
                                        op=ALU.add)
            else:
                r1 = outp.tile([128, 128], f32, tag="r1")
                nc.vector.tensor_tensor(out=r1[:], in0=hm[:],
                                        in1=biasB_sb[:], op=ALU.add)
                r2 = outp.tile([128, 128], f32, tag="r2")
                nc.vector.tensor_tensor(out=r2[:], in0=r1[:], in1=xt[:],
                                        op=ALU.add)
            mu = lgp.tile([128, 1], f32, tag="mu")
            nc.vector.tensor_reduce(out=mu[:], in_=r2[:], axis=X, op=ALU.add)
            mun = lgp.tile([128, 1], f32, tag="mun")
            nc.vector.tensor_scalar_mul(mun[:], mu[:], 1.0 / 128)
            xc = outp.tile([128, 128], f32, tag="xc")
            nc.vector.tensor_scalar(out=xc[:], in0=r2[:], scalar1=mun[:],
                                    scalar2=None, op0=ALU.subtract)
            junk = outp.tile([128, 128], f32, tag="junk")
            vs = lgp.tile([128, 1], f32, tag="vs")
            nc.vector.scalar_tensor_tensor(
                out=junk[:], in0=r2[:], scalar=mun[:], in1=xc[:],
                op0=ALU.subtract, op1=ALU.mult, accum_out=vs[:])
            varT = lgp.tile([128, 1], f32, tag="varT")
            nc.vector.tensor_scalar(out=varT[:], in0=vs[:],
                                    scalar1=1.0 / 128, scalar2=LN_EPS,
                                    op0=ALU.mult, op1=ALU.add)
            rvar = lgp.tile([128, 1], f32, tag="rvar")
            nc.vector.reciprocal(rvar[:], varT[:])
            rstd = lgp.tile([128, 1], f32, tag="rstd")
            nc.scalar.activation(rstd[:], rvar[:], AF.Sqrt)
            xn = outp.tile([128, 128], f32, tag="xn")
            nc.vector.tensor_scalar(out=xn[:], in0=xc[:], scalar1=rstd[:],
                                    scalar2=None, op0=ALU.mult)
            if ln_triv:
                xgb = xn
            else:
                xg = outp.tile([128, 128], f32, tag="xg")
                nc.vector.tensor_tensor(out=xg[:], in0=xn[:], in1=lngB_sb[:],
                                        op=ALU.mult)
                xgb = outp.tile([128, 128], f32, tag="xgb")
                nc.vector.tensor_tensor(out=xgb[:], in0=xg[:],
                                        in1=lnbB_sb[:], op=ALU.add)
            xout = outp.tile([128, 128], f32, tag="xout")
            nc.scalar.activation(xout[:], xgb[:], AF.Relu)
            nc.sync.dma_start(xnew[b * 128:(b + 1) * 128, :], xout[:])

        for i in range(NPAIR + 3):
            if i < NPAIR:
                conv_stage(*pairs[i])
            if 1 <= i and i - 1 < NPAIR:
                front(*pairs[i - 1])
            if 2 <= i and i - 2 < NPAIR:
                mid(*pairs[i - 2])
            if 3 <= i and i - 3 < NPAIR:
                back(*pairs[i - 3])

    nc.compile()
    return nc


def kernel(x, edge_index, Wl, bl, Wr, br, att, bias, ln_g, ln_b):
    x = np.asarray(x, np.float32)
    edge_index = np.asarray(edge_index)
    Wl = np.asarray(Wl, np.float32); bl = np.asarray(bl, np.float32)
    Wr = np.asarray(Wr, np.float32); br = np.asarray(br, np.float32)
    att = np.asarray(att, np.float32); bias = np.asarray(bias, np.float32)
    ln_g = np.asarray(ln_g, np.float32); ln_b = np.asarray(ln_b, np.float32)

    K, src_arr, dpos_arr = _prep_edges(edge_index)
    sidx, ohtid, ohflat = _build_ship_arrays(K, src_arr, dpos_arr)

    bias_zero = not (np.any(bias) or np.any(bl) or np.any(br))
    ln_triv = (np.all(ln_g == 1.0) and not np.any(ln_b))
    key = (K, bias_zero, ln_triv)
    if key not in _NC_CACHE:
        _NC_CACHE[key] = _build_nc(K, bias_zero, ln_triv)
    nc = _NC_CACHE[key]

    aatt = np.maximum(np.abs(att), 1e-30)
    sgn = np.sign(att).astype(np.float32)
    sgn[sgn == 0] = 1.0

    LAST_RESULTS.clear()
    cur = x
    for l in range(L):
        a_flat = aatt[l].reshape(HC)
        WlS = (Wl[l] * a_flat[None, :]).astype(BF16)
        WrS = (Wr[l] * a_flat[None, :]).astype(BF16)
        sgnT = np.ascontiguousarray(sgn[l].T).astype(BF16)   # [C, H]

        xpad = np.zeros((N_ROWS, 128), np.float32)
        xpad[:N_NODES] = cur
        xT = np.ascontiguousarray(xpad.T.astype(BF16))
        xloc_full = np.zeros((N_PAD, 128), np.float32)
        xloc_full[:N_NODES] = cur

        common = {
            "xT": xT, "WlS": WlS, "WrS": WrS,
            "blB": _bcast(bl[l] * a_flat), "brB": _bcast(br[l] * a_flat),
            "sgnT": sgnT, "invatt4B": _bcast(0.25 / a_flat),
            "biasB": _bcast(bias[l]), "lngB": _bcast(ln_g[l]),
            "lnbB": _bcast(ln_b[l]),
        }
        in_maps = []
        for c in range(N_CORES):
            xl_c = np.ascontiguousarray(
                xloc_full[c * NODES_PER_CORE:(c + 1) * NODES_PER_CORE])
            in_maps.append({
                **common,
                "xloc": xl_c,
                "xlocT": np.ascontiguousarray(xl_c.T.astype(BF16)),
                "ohtidd": ohtid[c], "ohd": ohflat[c], "sidxd": sidx[c],
            })

        res = run_bass_kernel_spmd(nc, in_maps, core_ids=list(range(N_CORES)))
        LAST_RESULTS.append(res)
        nxt = np.concatenate([res.results[c]["xnew"] for c in range(N_CORES)],
                             axis=0)
        cur = np.ascontiguousarray(nxt[:N_NODES])

    return cur.astype(np.float32)


# revision 6
# speedup vs baseline: 1.0000x; 1.0000x over previous
"""GATv2 (2 layers, H=4, C=128, head-mean) on 8 TRN2 cores, dst-partitioned.

v2 design (per layer, one SPMD launch of a shared single-layer NEFF):
  dense: xl'' = x @ (Wl .* |att|) for ALL nodes (bf16, to DRAM);
         xr'' per local block -> fp8 into slot 0 of the fp8 gather tile.
  per 128-edge chunk (edges sorted by dst, 10 blocks x 128 dst/core):
    - SWDGE row-gather xl''[src] (bf16) into slot k of the g tile
    - bf16->fp8 conversion of the gathered slot (round-robin Pool/DVE/ACT)
    - transposed u: per head one fp8 DoubleRow matmul computes
      uT[c,e] = sum_d xr[d,c]*oht[d,e] + g[e,c]  (both contractions packed
      as the two DoubleRow halves via slot-strided APs)
    - prelu on ACT -> s'T bf16
    - logits: per head a [128,1] matmul  z[e] = sum_c s'T[c,e]*sgn[c]
    - ez = e^z via DVE pow (f32), per chunk-pair
    - eg = g .* ez (4x tensor_scalar per chunk, bf16)
    - agg += oh^T @ eg, den += oh^T @ ez (PSUM accumulation over chunks)
  tail per block: alpha = agg*rden, .*(0.25/|att|), head-sum, +residual,
  LayerNorm (rstd = exp(-0.5*ln(var+eps))), relu.
Host: edge sorting, fp8 one-hot (oht|id) and oh arrays, wrapped gather idxs.
"""

from contextlib import ExitStack

import numpy as np
import ml_dtypes

import concourse.bacc as bacc
import concourse.tile as tile
from concourse import mybir
from concourse.bass_utils import run_bass_kernel_spmd

BF16 = ml_dtypes.bfloat16
FP8 = ml_dtypes.float8_e4m3fn

N_NODES = 10000
D = 128
H = 4
C = 128
HC = H * C
NEG_SLOPE = 0.2
LN_EPS = 1e-5
L = 2

N_CORES = 8
NODES_PER_CORE = 1280
BLOCKS = 10
BLK = 128
N_PAD = N_CORES * NODES_PER_CORE    # 10240
N_ROWS = 10112                      # 79*128
N_TILES = N_ROWS // 128

_NC_CACHE = {}
LAST_RESULTS = []   # BassKernelResults per launch (for test harness)

# conversion engine per chunk index: mostly Pool, some DVE, a little ACT
_CONV_PAT = ["ACT", "DVE", "POOL", "POOL", "DVE", "POOL",
             "POOL", "DVE", "POOL", "POOL", "DVE"]


def _prep_edges(edge_index):
    src = np.concatenate([np.asarray(edge_index[0], np.int64),
                          np.arange(N_NODES, dtype=np.int64)])
    dst = np.concatenate([np.asarray(edge_index[1], np.int64),
                          np.arange(N_NODES, dtype=np.int64)])
    pad_nodes = np.arange(N_NODES, N_PAD, dtype=np.int64)
    src = np.concatenate([src, np.zeros_like(pad_nodes)])
    dst = np.concatenate([dst, pad_nodes])

    order = np.argsort(dst, kind="stable")
    src = src[order]
    dst = dst[order]

    blk_of_edge = dst // BLK
    n_blocks_total = N_PAD // BLK
    counts = np.bincount(blk_of_edge, minlength=n_blocks_total)
    K = int(np.max((counts + BLK - 1) // BLK))
    K += K % 2  # even, so we can process chunk pairs

    cap = K * BLK
    src_arr = np.zeros((n_blocks_total, cap), np.int32)
    dpos_arr = np.full((n_blocks_total, cap), -1, np.int32)
    block_starts = np.zeros(n_blocks_total + 1, np.int64)
    np.cumsum(counts, out=block_starts[1:])
    slot = np.arange(len(dst)) - block_starts[blk_of_edge]
    src_arr[blk_of_edge, slot] = src.astype(np.int32)
    dpos_arr[blk_of_edge, slot] = (dst - blk_of_edge * BLK).astype(np.int32)

    return (K, src_arr.reshape(N_CORES, BLOCKS, cap),
            dpos_arr.reshape(N_CORES, BLOCKS, cap))


def _build_ship_arrays(K, src_arr, dpos_arr):
    cap = K * BLK
    # wrapped gather indices: idx i lives at [i % 16, i // 16]; the 16-row
    # pattern is tiled 8x along partitions (one copy per SWDGE Q7 core).
    s = src_arr.reshape(N_CORES, BLOCKS, cap // 16, 16)
    s = np.swapaxes(s, 2, 3)                                  # [c,b,16,cap/16]
    sidx = np.tile(s, (1, 1, 8, 1)).astype(np.int16)          # [c,b,128,cap/16]

    # fp8 one-hots:
    # ohtid [c,b, d(128), (K+1)*128]: slot k col k*128+e -> oht[d,e]=1 iff
    #   dst(chunk k, e) == d; slot K = identity (rows e).
    # ohflat [c,b, e(128), cap]: col k*128+d -> oh[e, k, d]
    ohtid = np.zeros((N_CORES, BLOCKS, BLK, (K + 1) * BLK), FP8)
    ohflat = np.zeros((N_CORES, BLOCKS, BLK, cap), FP8)
    cc, bb, ss = np.nonzero(dpos_arr >= 0)
    kk = (ss // BLK).astype(np.int64)
    ee = (ss % BLK).astype(np.int64)
    dd = dpos_arr[cc, bb, ss].astype(np.int64)
    ohtid[cc, bb, dd, kk * BLK + ee] = 1
    ohflat[cc, bb, ee, kk * BLK + dd] = 1
    i = np.arange(BLK)
    ohtid[:, :, i, K * BLK + i] = 1
    return (np.ascontiguousarray(sidx), np.ascontiguousarray(ohtid),
            np.ascontiguousarray(ohflat))


def _bcast(v, rows=128):
    v = np.asarray(v, np.float32)
    return np.ascontiguousarray(np.broadcast_to(v[None, :], (rows, v.shape[0])))


def _build_nc(K, bias_zero, ln_triv):
    nc = bacc.Bacc("TRN2", target_bir_lowering=False, debug=False,
                   num_devices=N_CORES)
    f32, bf16, i16 = mybir.dt.float32, mybir.dt.bfloat16, mybir.dt.int16
    fp8 = mybir.dt.float8e4
    AF = mybir.ActivationFunctionType
    ALU = mybir.AluOpType
    PM = mybir.MatmulPerfMode
    X = mybir.AxisListType.X
    cap = K * BLK

    xT = nc.dram_tensor("xT", [128, N_ROWS], bf16, kind="ExternalInput")
    xlocT = nc.dram_tensor("xlocT", [128, NODES_PER_CORE], bf16,
                           kind="ExternalInput")
    xloc = nc.dram_tensor("xloc", [NODES_PER_CORE, 128], f32,
                          kind="ExternalInput")
    WlS = nc.dram_tensor("WlS", [128, HC], bf16, kind="ExternalInput")
    WrS = nc.dram_tensor("WrS", [128, HC], bf16, kind="ExternalInput")
    blB = nc.dram_tensor("blB", [128, HC], f32, kind="ExternalInput")
    brB = nc.dram_tensor("brB", [128, HC], f32, kind="ExternalInput")
    sgnT = nc.dram_tensor("sgnT", [128, H], bf16, kind="ExternalInput")
    invatt4B = nc.dram_tensor("invatt4B", [128, HC], f32, kind="ExternalInput")
    biasB = nc.dram_tensor("biasB", [128, 128], f32, kind="ExternalInput")
    lngB = nc.dram_tensor("lngB", [128, 128], f32, kind="ExternalInput")
    lnbB = nc.dram_tensor("lnbB", [128, 128], f32, kind="ExternalInput")
    ohtidd = nc.dram_tensor("ohtidd", [BLOCKS, BLK, (K + 1) * BLK], fp8,
                            kind="ExternalInput")
    ohd = nc.dram_tensor("ohd", [BLOCKS, BLK, cap], fp8, kind="ExternalInput")
    sidxd = nc.dram_tensor("sidxd", [BLOCKS, 128, cap // 16], i16,
                           kind="ExternalInput")

    xnew = nc.dram_tensor("xnew", [NODES_PER_CORE, 128], f32,
                          kind="ExternalOutput")

    with tile.TileContext(nc) as tc, ExitStack() as ctx:
        consts = ctx.enter_context(tc.tile_pool(name="consts", bufs=1))
        lhsp = ctx.enter_context(tc.tile_pool(name="lhs", bufs=3))
        densep = ctx.enter_context(tc.tile_pool(name="dense", bufs=3))
        gp = ctx.enter_context(tc.tile_pool(name="g", bufs=2))
        g8p = ctx.enter_context(tc.tile_pool(name="g8", bufs=2))
        otp = ctx.enter_context(tc.tile_pool(name="ot", bufs=2))
        ohp = ctx.enter_context(tc.tile_pool(name="ohf", bufs=2))
        sxp = ctx.enter_context(tc.tile_pool(name="sx", bufs=2))
        sp = ctx.enter_context(tc.tile_pool(name="s", bufs=3))
        ezp = ctx.enter_context(tc.tile_pool(name="ez", bufs=4))
        egp = ctx.enter_context(tc.tile_pool(name="eg", bufs=3))
        lnp = ctx.enter_context(tc.tile_pool(name="ln", bufs=2))
        lgp = ctx.enter_context(tc.tile_pool(name="lg", bufs=4))
        outp = ctx.enter_context(tc.tile_pool(name="out", bufs=2))
        dramp = ctx.enter_context(tc.tile_pool(name="dram", bufs=1,
                                               space="DRAM"))
        pup = ctx.enter_context(tc.tile_pool(name="pu", bufs=2, space="PSUM"))
        pzp = ctx.enter_context(tc.tile_pool(name="pz", bufs=1, space="PSUM"))
        pdenp = ctx.enter_context(tc.tile_pool(name="pden", bufs=1,
                                               space="PSUM"))
        paggp = ctx.enter_context(tc.tile_pool(name="pagg", bufs=2,
                                               space="PSUM"))

        def load_const(src_ap, shape, dtype, name):
            t = consts.tile(shape, dtype, tag=name)
            nc.sync.dma_start(t[:], src_ap)
            return t

        wl_sb = load_const(WlS[:], [128, HC], bf16, "wl")
        wr_sb = load_const(WrS[:], [128, HC], bf16, "wr")
        sgn_sb = load_const(sgnT[:], [128, H], bf16, "sgn")
        invatt_sb = load_const(invatt4B[:], [128, HC], f32, "invatt")
        if not bias_zero:
            blB_sb = load_const(blB[:], [128, HC], f32, "blB")
            brB_sb = load_const(brB[:], [128, HC], f32, "brB")
            biasB_sb = load_const(biasB[:], [128, 128], f32, "biasB")
        if not ln_triv:
            lngB_sb = load_const(lngB[:], [128, 128], f32, "lngB")
            lnbB_sb = load_const(lnbB[:], [128, 128], f32, "lnbB")

        xl_dram = dramp.tile([N_ROWS, HC], bf16)

        alphaP = consts.tile([128, 1], f32, tag="alphaP")
        nc.vector.memset(alphaP[:], NEG_SLOPE)

        blk_loads = {}
        blk_g = {}

        def prefetch_loads(b):
            six = sxp.tile([128, cap // 16], i16, tag="sidx")
            nc.sync.dma_start(six[:], sidxd[b])
            ot = otp.tile([128, K + 1, BLK], fp8, tag="ot")
            nc.sync.dma_start(
                ot[:], ohtidd[b].rearrange("p (k e) -> p k e", e=BLK))
            ohb = ohp.tile([128, cap], fp8, tag="oh")
            nc.sync.dma_start(ohb[:], ohd[b])
            blk_loads[b] = (six, ot, ohb)

        def prefetch_gather(b, splits=3):
            six, ot, ohb = blk_loads[b]
            g = gp.tile([128, K + 1, HC], bf16, tag="g")
            bounds = [K * i // splits for i in range(splits + 1)]
            for i in range(splits):
                k0, k1 = bounds[i], bounds[i + 1]
                n_idx = (k1 - k0) * BLK
                nc.gpsimd.dma_gather(
                    out_ap=g[:, 1 + k0:1 + k1, :], in_ap=xl_dram[:],
                    idxs_ap=six[:, k0 * BLK // 16:k1 * BLK // 16],
                    num_idxs=n_idx, num_idxs_reg=n_idx, elem_size=HC,
                    single_packet=False)
            blk_g[b] = g

        # ---- dense: xl'' for all nodes -> DRAM bf16 ----
        # single batched xT load; xl_dram stores batched 4 tiles at a time
        xT_sb = consts.tile([128, N_ROWS], bf16, tag="xT")
        for q in range(4):
            c0 = (N_ROWS // 4 // 128) * 128 * q
            c1 = N_ROWS if q == 3 else (N_ROWS // 4 // 128) * 128 * (q + 1)
            nc.sync.dma_start(xT_sb[:, c0:c1], xT[:, c0:c1])
        prefetch_loads(0)
        GB = 4
        for t0 in range(0, N_TILES, GB):
            n_sub = min(GB, N_TILES - t0)
            xs4 = densep.tile([128, GB, HC], bf16, tag="xs4")
            for j in range(n_sub):
                t_i = t0 + j
                if j % 2 == 0:
                    ps = pup.tile([128, 2, HC], f32, tag="uT")
                nc.tensor.matmul(ps[:, j % 2, :],
                                 xT_sb[:, t_i * 128:(t_i + 1) * 128],
                                 wl_sb[:], start=True, stop=True)
                if bias_zero:
                    if t_i % 2 == 0:
                        nc.scalar.activation(xs4[:, j, :], ps[:, j % 2, :],
                                             AF.Copy)
                    else:
                        nc.vector.tensor_scalar(out=xs4[:, j, :],
                                                in0=ps[:, j % 2, :],
                                                scalar1=1.0, scalar2=None,
                                                op0=ALU.mult)
                else:
                    nc.vector.tensor_tensor(out=xs4[:, j, :],
                                            in0=ps[:, j % 2, :],
                                            in1=blB_sb[:], op=ALU.add)
            nc.sync.dma_start(
                xl_dram[t0 * 128:(t0 + n_sub) * 128, :].rearrange(
                    "(t p) c -> p t c", p=128),
                xs4[:, :n_sub, :])

        # ---- edge: flattened cross-block software pipeline ----
        # Global pair list; stage offsets (conv @i, u+prelu @i-1, z/exp/eg
        # @i-2, agg/den @i-3) run across block boundaries so no engine
        # queue drains at block edges. Block tails are emitted right after
        # the block's last agg/den.
        prefetch_gather(0, splits=6)
        P = K // 2
        pairs = [(b, p) for b in range(BLOCKS) for p in range(P)]
        NPAIR = len(pairs)
        st = {"g8": {}, "agg": {}, "den": {}, "s": {}, "ez": {}, "eg": {}}

        def blk_state(b):
            if b not in st["g8"]:
                g8 = g8p.tile([128, K + 1, HC], fp8, tag="g8")
                st["g8"][b] = g8
                # xr'' for this block -> fp8 slot 0
                lhs = lhsp.tile([128, 128], bf16, tag="lhs")
                nc.sync.dma_start(lhs[:], xlocT[:, b * 128:(b + 1) * 128])
                psr2 = pup.tile([128, 2, HC], f32, tag="uT")
                psr = psr2[:, 0, :]
                nc.tensor.matmul(psr, lhs[:], wr_sb[:], start=True, stop=True)
                if bias_zero:
                    nc.scalar.activation(g8[:, 0, :], psr, AF.Copy)
                else:
                    xrt = densep.tile([128, HC], bf16, tag="xrt")
                    nc.vector.tensor_tensor(out=xrt[:], in0=psr,
                                            in1=brB_sb[:], op=ALU.add)
                    nc.scalar.activation(g8[:, 0, :], xrt[:], AF.Copy)
                agg_t = paggp.tile([128, HC], f32, tag="agg")
                den_t = pdenp.tile([128, 4], f32, tag="den")
                st["agg"][b] = agg_t
                st["den"][b] = den_t
            return st["g8"][b], st["agg"][b], st["den"][b]

        def conv_stage(b, p):
            g8, _, _ = blk_state(b)
            g = blk_g[b]
            for j in (0, 1):
                slot = 2 * p + j + 1
                nc.vector.tensor_scalar(
                    out=g8[:, slot, :], in0=g[:, slot, :],
                    scalar1=1.0, scalar2=None, op0=ALU.mult)
            if p == 15 and b + 1 < BLOCKS:
                prefetch_loads(b + 1)
            if p == 16 and b + 1 < BLOCKS:
                prefetch_gather(b + 1)

        def front(b, p):
            g8, _, _ = blk_state(b)
            ot = blk_loads[b][1]
            uT = pup.tile([128, 2, HC], f32, tag="uT")
            for j in (0, 1):
                k = 2 * p + j
                slot = k + 1
                for h in range(H):
                    lhs_ap = g8[:, 0:slot + 1:slot, h * 128:(h + 1) * 128]
                    rhs_ap = ot[:, k:K + 1:K - k, :]
                    nc.tensor.matmul(uT[:, j, h * 128:(h + 1) * 128],
                                     lhs_ap, rhs_ap, start=True, stop=True,
                                     perf_mode=PM.DoubleRow)
            s_ = sp.tile([128, 2, HC], bf16, tag="s")
            nc.scalar.activation(s_[:], uT[:], AF.Prelu, alpha=alphaP[:])
            st["s"][(b, p)] = s_

        def mid(b, p):
            g = blk_g[b]
            zP = pzp.tile([128, 8], f32, tag="zP")
            s_ = st["s"].pop((b, p))
            for j in (0, 1):
                for h in range(H):
                    nc.tensor.matmul(zP[:, j * 4 + h:j * 4 + h + 1],
                                     s_[:, j, h * 128:(h + 1) * 128],
                                     sgn_sb[:, h:h + 1],
                                     start=True, stop=True)
            ezf = ezp.tile([128, 8], f32, tag="ez")
            nc.scalar.activation(ezf[:], zP[:], AF.Exp)
            ezb = ezp.tile([128, 8], bf16, tag="ezb")
            nc.vector.tensor_scalar(out=ezb[:], in0=ezf[:], scalar1=1.0,
                                    scalar2=None, op0=ALU.mult)
            st["ez"][(b, p)] = ezb
            eg = egp.tile([128, 2, HC], bf16, tag="eg")
            for j in (0, 1):
                k = 2 * p + j
                for h in range(H):
                    nc.gpsimd.tensor_scalar(
                        out=eg[:, j, h * 128:(h + 1) * 128],
                        in0=g[:, k + 1, h * 128:(h + 1) * 128],
                        scalar1=ezf[:, j * 4 + h:j * 4 + h + 1],
                        scalar2=None, op0=ALU.mult)
            st["eg"][(b, p)] = eg

        def back(b, p):
            _, agg, den = blk_state(b)
            ohb = blk_loads[b][2]
            eg = st["eg"].pop((b, p))
            ezb = st["ez"].pop((b, p))
            for j in (0, 1):
                k = 2 * p + j
                nc.tensor.matmul(agg[:], ohb[:, k * BLK:(k + 1) * BLK],
                                 eg[:, j, :],
                                 start=(k == 0), stop=(k == K - 1))
                nc.tensor.matmul(den[:], ohb[:, k * BLK:(k + 1) * BLK],
                                 ezb[:, j * 4:(j + 1) * 4],
                                 start=(k == 0), stop=(k == K - 1))
            if p == P - 1:
                tail(b)

        def tail(b):
            agg = st["agg"].pop(b)
            den = st["den"].pop(b)
            for dd in ("g8",):
                st[dd].pop(b, None)
            blk_g.pop(b, None)
            blk_loads.pop(b, None)
            rden = lgp.tile([128, 4], f32, tag="rden")
            nc.vector.reciprocal(rden[:], den[:])
            tq = lnp.tile([128, HC], f32, tag="tq")
            for h in range(H):
                nc.vector.tensor_scalar(
                    out=tq[:, h * 128:(h + 1) * 128],
                    in0=agg[:, h * 128:(h + 1) * 128],
                    scalar1=rden[:, h:h + 1], scalar2=None, op0=ALU.mult)
            tq2 = lnp.tile([128, HC], f32, tag="tq2")
            nc.vector.tensor_tensor(out=tq2[:], in0=tq[:], in1=invatt_sb[:],
                                    op=ALU.mult)
            hm = outp.tile([128, 128], f32, tag="hm")
            nc.vector.tensor_reduce(
                out=hm[:], in_=tq2[:].rearrange("p (h c) -> p c h", h=H),
                axis=X, op=ALU.add)
            xt = outp.tile([128, 128], f32, tag="xres")
            nc.sync.dma_start(xt[:], xloc[b * 128:(b + 1) * 128, :])
            if bias_zero:
                r2 = outp.tile([128, 128], f32, tag="r2")
                nc.vector.tensor_tensor(out=r2[:], in0=hm[:], in1=xt[:],
                                        op=ALU.add)
            else:
                r1 = outp.tile([128, 128], f32, tag="r1")
                nc.vector.tensor_tensor(out=r1[:], in0=hm[:],
                                        in1=biasB_sb[:], op=ALU.add)
                r2 = outp.tile([128, 128], f32, tag="r2")
                nc.vector.tensor_tensor(out=r2[:], in0=r1[:], in1=xt[:],
                                        op=ALU.add)
            mu = lgp.tile([128, 1], f32, tag="mu")
            nc.vector.tensor_reduce(out=mu[:], in_=r2[:], axis=X, op=ALU.add)
            mun = lgp.tile([128, 1], f32, tag="mun")
            nc.vector.tensor_scalar_mul(mun[:], mu[:], 1.0 / 128)
            xc = outp.tile([128, 128], f32, tag="xc")
            nc.vector.tensor_scalar(out=xc[:], in0=r2[:], scalar1=mun[:],
                                    scalar2=None, op0=ALU.subtract)
            junk = outp.tile([128, 128], f32, tag="junk")
            vs = lgp.tile([128, 1], f32, tag="vs")
            nc.vector.scalar_tensor_tensor(
                out=junk[:], in0=r2[:], scalar=mun[:], in1=xc[:],
                op0=ALU.subtract, op1=ALU.mult, accum_out=vs[:])
            varT = lgp.tile([128, 1], f32, tag="varT")
            nc.vector.tensor_scalar(out=varT[:], in0=vs[:],
                                    scalar1=1.0 / 128, scalar2=LN_EPS,
                                    op0=ALU.mult, op1=ALU.add)
            rvar = lgp.tile([128, 1], f32, tag="rvar")
            nc.vector.reciprocal(rvar[:], varT[:])
            rstd = lgp.tile([128, 1], f32, tag="rstd")
            nc.scalar.activation(rstd[:], rvar[:], AF.Sqrt)
            xn = outp.tile([128, 128], f32, tag="xn")
            nc.vector.tensor_scalar(out=xn[:], in0=xc[:], scalar1=rstd[:],
                                    scalar2=None, op0=ALU.mult)
            if ln_triv:
                xgb = xn
            else:
                xg = outp.tile([128, 128], f32, tag="xg")
                nc.vector.tensor_tensor(out=xg[:], in0=xn[:], in1=lngB_sb[:],
                                        op=ALU.mult)
                xgb = outp.tile([128, 128], f32, tag="xgb")
                nc.vector.tensor_tensor(out=xgb[:], in0=xg[:],
                                        in1=lnbB_sb[:], op=ALU.add)
            xout = outp.tile([128, 128], f32, tag="xout")
            nc.scalar.activation(xout[:], xgb[:], AF.Relu)
            nc.sync.dma_start(xnew[b * 128:(b + 1) * 128, :], xout[:])

        for i in range(NPAIR + 3):
            if i < NPAIR:
                conv_stage(*pairs[i])
            if 1 <= i and i - 1 < NPAIR:
                front(*pairs[i - 1])
            if 2 <= i and i - 2 < NPAIR:
                mid(*pairs[i - 2])
            if 3 <= i and i - 3 < NPAIR:
                back(*pairs[i - 3])

    nc.compile()
    return nc


def kernel(x, edge_index, Wl, bl, Wr, br, att, bias, ln_g, ln_b):
    x = np.asarray(x, np.float32)
    edge_index = np.asarray(edge_index)
    Wl = np.asarray(Wl, np.float32); bl = np.asarray(bl, np.float32)
    Wr = np.asarray(Wr, np.float32); br = np.asarray(br, np.float32)
    att = np.asarray(att, np.float32); bias = np.asarray(bias, np.float32)
    ln_g = np.asarray(ln_g, np.float32); ln_b = np.asarray(ln_b, np.float32)

    K, src_arr, dpos_arr = _prep_edges(edge_index)
    sidx, ohtid, ohflat = _build_ship_arrays(K, src_arr, dpos_arr)

    bias_zero = not (np.any(bias) or np.any(bl) or np.any(br))
    ln_triv = (np.all(ln_g == 1.0) and not np.any(ln_b))
    key = (K, bias_zero, ln_triv)
    if key not in _NC_CACHE:
        _NC_CACHE[key] = _build_nc(K, bias_zero, ln_triv)
    nc = _NC_CACHE[key]

    aatt = np.maximum(np.abs(att), 1e-30)
    sgn = np.sign(att).astype(np.float32)
    sgn[sgn == 0] = 1.0

    LAST_RESULTS.clear()
    cur = x
    for l in range(L):
        a_flat = aatt[l].reshape(HC)
        WlS = (Wl[l] * a_flat[None, :]).astype(BF16)
        WrS = (Wr[l] * a_flat[None, :]).astype(BF16)
        sgnT = np.ascontiguousarray(sgn[l].T).astype(BF16)   # [C, H]

        xpad = np.zeros((N_ROWS, 128), np.float32)
        xpad[:N_NODES] = cur
        xT = np.ascontiguousarray(xpad.T.astype(BF16))
        xloc_full = np.zeros((N_PAD, 128), np.float32)
        xloc_full[:N_NODES] = cur

        common = {
            "xT": xT, "WlS": WlS, "WrS": WrS,
            "blB": _bcast(bl[l] * a_flat), "brB": _bcast(br[l] * a_flat),
            "sgnT": sgnT, "invatt4B": _bcast(0.25 / a_flat),
            "biasB": _bcast(bias[l]), "lngB": _bcast(ln_g[l]),
            "lnbB": _bcast(ln_b[l]),
        }
        in_maps = []
        for c in range(N_CORES):
            xl_c = np.ascontiguousarray(
                xloc_full[c * NODES_PER_CORE:(c + 1) * NODES_PER_CORE])
            in_maps.append({
                **common,
                "xloc": xl_c,
                "xlocT": np.ascontiguousarray(xl_c.T.astype(BF16)),
                "ohtidd": ohtid[c], "ohd": ohflat[c], "sidxd": sidx[c],
            })

        res = run_bass_kernel_spmd(nc, in_maps, core_ids=list(range(N_CORES)))
        LAST_RESULTS.append(res)
        nxt = np.concatenate([res.results[c]["xnew"] for c in range(N_CORES)],
                             axis=0)
        cur = np.ascontiguousarray(nxt[:N_NODES])

    return cur.astype(np.float32)


# revision 7
# speedup vs baseline: 1.0011x; 1.0010x over previous
"""GATv2 (2 layers, H=4, C=128, head-mean) on 8 TRN2 cores, dst-partitioned.

v2 design (per layer, one SPMD launch of a shared single-layer NEFF):
  dense: xl'' = x @ (Wl .* |att|) for ALL nodes (bf16, to DRAM);
         xr'' per local block -> fp8 into slot 0 of the fp8 gather tile.
  per 128-edge chunk (edges sorted by dst, 10 blocks x 128 dst/core):
    - SWDGE row-gather xl''[src] (bf16) into slot k of the g tile
    - bf16->fp8 conversion of the gathered slot (round-robin Pool/DVE/ACT)
    - transposed u: per head one fp8 DoubleRow matmul computes
      uT[c,e] = sum_d xr[d,c]*oht[d,e] + g[e,c]  (both contractions packed
      as the two DoubleRow halves via slot-strided APs)
    - prelu on ACT -> s'T bf16
    - logits: per head a [128,1] matmul  z[e] = sum_c s'T[c,e]*sgn[c]
    - ez = e^z on ACT (f32), per chunk-pair
    - eg = g .* ez (gpsimd tensor_scalar per head, bf16)
    - agg += oh^T @ eg, den += oh^T @ ez (PSUM accumulation over chunks)
  All stages run in a flattened cross-block software pipeline (conv @i,
  u+prelu @i-1, z/exp/eg @i-2, agg/den @i-3) so the in-order engine queues
  never drain at block boundaries.
  tail per block: alpha = agg*rden, .*(0.25/|att|), head-sum, +residual,
  LayerNorm (rstd = sqrt(1/(var+eps))), relu.
Host: edge sorting, fp8 one-hot (oht|id) and oh arrays, wrapped gather idxs.
"""

from contextlib import ExitStack

import numpy as np
import ml_dtypes

import concourse.bacc as bacc
import concourse.tile as tile
from concourse import mybir
from concourse.bass_utils import run_bass_kernel_spmd

BF16 = ml_dtypes.bfloat16
FP8 = ml_dtypes.float8_e4m3fn

N_NODES = 10000
D = 128
H = 4
C = 128
HC = H * C
NEG_SLOPE = 0.2
LN_EPS = 1e-5
L = 2

N_CORES = 8
NODES_PER_CORE = 1280
BLOCKS = 10
BLK = 128
N_PAD = N_CORES * NODES_PER_CORE    # 10240
N_ROWS = 10112                      # 79*128
N_TILES = N_ROWS // 128

_NC_CACHE = {}
LAST_RESULTS = []   # BassKernelResults per launch (for test harness)

# conversion engine per chunk index: mostly Pool, some DVE, a little ACT
_CONV_PAT = ["ACT", "DVE", "POOL", "POOL", "DVE", "POOL",
             "POOL", "DVE", "POOL", "POOL", "DVE"]


def _prep_edges(edge_index):
    src = np.concatenate([np.asarray(edge_index[0], np.int64),
                          np.arange(N_NODES, dtype=np.int64)])
    dst = np.concatenate([np.asarray(edge_index[1], np.int64),
                          np.arange(N_NODES, dtype=np.int64)])
    pad_nodes = np.arange(N_NODES, N_PAD, dtype=np.int64)
    src = np.concatenate([src, np.zeros_like(pad_nodes)])
    dst = np.concatenate([dst, pad_nodes])

    order = np.argsort(dst, kind="stable")
    src = src[order]
    dst = dst[order]

    blk_of_edge = dst // BLK
    n_blocks_total = N_PAD // BLK
    counts = np.bincount(blk_of_edge, minlength=n_blocks_total)
    K = int(np.max((counts + BLK - 1) // BLK))
    K += K % 2  # even, so we can process chunk pairs

    cap = K * BLK
    src_arr = np.zeros((n_blocks_total, cap), np.int32)
    dpos_arr = np.full((n_blocks_total, cap), -1, np.int32)
    block_starts = np.zeros(n_blocks_total + 1, np.int64)
    np.cumsum(counts, out=block_starts[1:])
    slot = np.arange(len(dst)) - block_starts[blk_of_edge]
    src_arr[blk_of_edge, slot] = src.astype(np.int32)
    dpos_arr[blk_of_edge, slot] = (dst - blk_of_edge * BLK).astype(np.int32)

    return (K, src_arr.reshape(N_CORES, BLOCKS, cap),
            dpos_arr.reshape(N_CORES, BLOCKS, cap))


def _build_ship_arrays(K, src_arr, dpos_arr):
    cap = K * BLK
    # wrapped gather indices: idx i lives at [i % 16, i // 16]; the 16-row
    # pattern is tiled 8x along partitions (one copy per SWDGE Q7 core).
    s = src_arr.reshape(N_CORES, BLOCKS, cap // 16, 16)
    s = np.swapaxes(s, 2, 3)                                  # [c,b,16,cap/16]
    sidx = np.tile(s, (1, 1, 8, 1)).astype(np.int16)          # [c,b,128,cap/16]

    # fp8 one-hots:
    # ohtid [c,b, d(128), (K+1)*128]: slot k col k*128+e -> oht[d,e]=1 iff
    #   dst(chunk k, e) == d; slot K = identity (rows e).
    # ohflat [c,b, e(128), cap]: col k*128+d -> oh[e, k, d]
    ohtid = np.zeros((N_CORES, BLOCKS, BLK, (K + 1) * BLK), FP8)
    ohflat = np.zeros((N_CORES, BLOCKS, BLK, cap), FP8)
    cc, bb, ss = np.nonzero(dpos_arr >= 0)
    kk = (ss // BLK).astype(np.int64)
    ee = (ss % BLK).astype(np.int64)
    dd = dpos_arr[cc, bb, ss].astype(np.int64)
    ohtid[cc, bb, dd, kk * BLK + ee] = 1
    ohflat[cc, bb, ee, kk * BLK + dd] = 1
    i = np.arange(BLK)
    ohtid[:, :, i, K * BLK + i] = 1
    return (np.ascontiguousarray(sidx), np.ascontiguousarray(ohtid),
            np.ascontiguousarray(ohflat))


def _bcast(v, rows=128):
    v = np.asarray(v, np.float32)
    return np.ascontiguousarray(np.broadcast_to(v[None, :], (rows, v.shape[0])))


def _build_nc(K, bias_zero, ln_triv):
    nc = bacc.Bacc("TRN2", target_bir_lowering=False, debug=False,
                   num_devices=N_CORES)
    f32, bf16, i16 = mybir.dt.float32, mybir.dt.bfloat16, mybir.dt.int16
    fp8 = mybir.dt.float8e4
    AF = mybir.ActivationFunctionType
    ALU = mybir.AluOpType
    PM = mybir.MatmulPerfMode
    X = mybir.AxisListType.X
    cap = K * BLK

    xT = nc.dram_tensor("xT", [128, N_ROWS], bf16, kind="ExternalInput")
    xlocT = nc.dram_tensor("xlocT", [128, NODES_PER_CORE], bf16,
                           kind="ExternalInput")
    xloc = nc.dram_tensor("xloc", [NODES_PER_CORE, 128], f32,
                          kind="ExternalInput")
    WlS = nc.dram_tensor("WlS", [128, HC], bf16, kind="ExternalInput")
    WrS = nc.dram_tensor("WrS", [128, HC], bf16, kind="ExternalInput")
    blB = nc.dram_tensor("blB", [128, HC], f32, kind="ExternalInput")
    brB = nc.dram_tensor("brB", [128, HC], f32, kind="ExternalInput")
    sgnT = nc.dram_tensor("sgnT", [128, H], bf16, kind="ExternalInput")
    invatt4B = nc.dram_tensor("invatt4B", [128, HC], f32, kind="ExternalInput")
    biasB = nc.dram_tensor("biasB", [128, 128], f32, kind="ExternalInput")
    lngB = nc.dram_tensor("lngB", [128, 128], f32, kind="ExternalInput")
    lnbB = nc.dram_tensor("lnbB", [128, 128], f32, kind="ExternalInput")
    ohtidd = nc.dram_tensor("ohtidd", [BLOCKS, BLK, (K + 1) * BLK], fp8,
                            kind="ExternalInput")
    ohd = nc.dram_tensor("ohd", [BLOCKS, BLK, cap], fp8, kind="ExternalInput")
    sidxd = nc.dram_tensor("sidxd", [BLOCKS, 128, cap // 16], i16,
                           kind="ExternalInput")

    xnew = nc.dram_tensor("xnew", [NODES_PER_CORE, 128], f32,
                          kind="ExternalOutput")

    with tile.TileContext(nc) as tc, ExitStack() as ctx:
        consts = ctx.enter_context(tc.tile_pool(name="consts", bufs=1))
        lhsp = ctx.enter_context(tc.tile_pool(name="lhs", bufs=3))
        densep = ctx.enter_context(tc.tile_pool(name="dense", bufs=3))
        gp = ctx.enter_context(tc.tile_pool(name="g", bufs=2))
        g8p = ctx.enter_context(tc.tile_pool(name="g8", bufs=2))
        otp = ctx.enter_context(tc.tile_pool(name="ot", bufs=2))
        ohp = ctx.enter_context(tc.tile_pool(name="ohf", bufs=2))
        sxp = ctx.enter_context(tc.tile_pool(name="sx", bufs=2))
        sp = ctx.enter_context(tc.tile_pool(name="s", bufs=3))
        ezp = ctx.enter_context(tc.tile_pool(name="ez", bufs=4))
        egp = ctx.enter_context(tc.tile_pool(name="eg", bufs=3))
        lnp = ctx.enter_context(tc.tile_pool(name="ln", bufs=2))
        lgp = ctx.enter_context(tc.tile_pool(name="lg", bufs=4))
        outp = ctx.enter_context(tc.tile_pool(name="out", bufs=2))
        dramp = ctx.enter_context(tc.tile_pool(name="dram", bufs=1,
                                               space="DRAM"))
        pup = ctx.enter_context(tc.tile_pool(name="pu", bufs=2, space="PSUM"))
        pzp = ctx.enter_context(tc.tile_pool(name="pz", bufs=1, space="PSUM"))
        pdenp = ctx.enter_context(tc.tile_pool(name="pden", bufs=1,
                                               space="PSUM"))
        paggp = ctx.enter_context(tc.tile_pool(name="pagg", bufs=2,
                                               space="PSUM"))

        def load_const(src_ap, shape, dtype, name):
            t = consts.tile(shape, dtype, tag=name)
            nc.sync.dma_start(t[:], src_ap)
            return t

        wl_sb = load_const(WlS[:], [128, HC], bf16, "wl")
        wr_sb = load_const(WrS[:], [128, HC], bf16, "wr")
        sgn_sb = load_const(sgnT[:], [128, H], bf16, "sgn")
        invatt_sb = load_const(invatt4B[:], [128, HC], f32, "invatt")
        if not bias_zero:
            blB_sb = load_const(blB[:], [128, HC], f32, "blB")
            brB_sb = load_const(brB[:], [128, HC], f32, "brB")
            biasB_sb = load_const(biasB[:], [128, 128], f32, "biasB")
        if not ln_triv:
            lngB_sb = load_const(lngB[:], [128, 128], f32, "lngB")
            lnbB_sb = load_const(lnbB[:], [128, 128], f32, "lnbB")

        xl_dram = dramp.tile([N_ROWS, HC], bf16)

        alphaP = consts.tile([128, 1], f32, tag="alphaP")
        nc.vector.memset(alphaP[:], NEG_SLOPE)

        blk_loads = {}
        blk_g = {}

        def prefetch_loads(b):
            six = sxp.tile([128, cap // 16], i16, tag="sidx")
            nc.sync.dma_start(six[:], sidxd[b])
            ot = otp.tile([128, K + 1, BLK], fp8, tag="ot")
            nc.sync.dma_start(
                ot[:], ohtidd[b].rearrange("p (k e) -> p k e", e=BLK))
            ohb = ohp.tile([128, cap], fp8, tag="oh")
            nc.sync.dma_start(ohb[:], ohd[b])
            blk_loads[b] = (six, ot, ohb)

        def prefetch_gather(b, splits=3):
            six, ot, ohb = blk_loads[b]
            g = gp.tile([128, K + 1, HC], bf16, tag="g")
            bounds = [K * i // splits for i in range(splits + 1)]
            for i in range(splits):
                k0, k1 = bounds[i], bounds[i + 1]
                n_idx = (k1 - k0) * BLK
                nc.gpsimd.dma_gather(
                    out_ap=g[:, 1 + k0:1 + k1, :], in_ap=xl_dram[:],
                    idxs_ap=six[:, k0 * BLK // 16:k1 * BLK // 16],
                    num_idxs=n_idx, num_idxs_reg=n_idx, elem_size=HC,
                    single_packet=False)
            blk_g[b] = g

        # ---- dense: xl'' for all nodes -> DRAM bf16 ----
        # single batched xT load; xl_dram stores batched 4 tiles at a time
        xT_sb = consts.tile([128, N_ROWS], bf16, tag="xT")
        for q in range(4):
            c0 = (N_ROWS // 4 // 128) * 128 * q
            c1 = N_ROWS if q == 3 else (N_ROWS // 4 // 128) * 128 * (q + 1)
            nc.sync.dma_start(xT_sb[:, c0:c1], xT[:, c0:c1])
        prefetch_loads(0)
        GB = 4
        for t0 in range(0, N_TILES, GB):
            n_sub = min(GB, N_TILES - t0)
            xs4 = densep.tile([128, GB, HC], bf16, tag="xs4")
            for j in range(n_sub):
                t_i = t0 + j
                if j % 2 == 0:
                    ps = pup.tile([128, 2, HC], f32, tag="uT")
                nc.tensor.matmul(ps[:, j % 2, :],
                                 xT_sb[:, t_i * 128:(t_i + 1) * 128],
                                 wl_sb[:], start=True, stop=True)
                if bias_zero:
                    if t_i % 2 == 0:
                        nc.scalar.activation(xs4[:, j, :], ps[:, j % 2, :],
                                             AF.Copy)
                    else:
                        nc.vector.tensor_scalar(out=xs4[:, j, :],
                                                in0=ps[:, j % 2, :],
                                                scalar1=1.0, scalar2=None,
                                                op0=ALU.mult)
                else:
                    nc.vector.tensor_tensor(out=xs4[:, j, :],
                                            in0=ps[:, j % 2, :],
                                            in1=blB_sb[:], op=ALU.add)
            nc.sync.dma_start(
                xl_dram[t0 * 128:(t0 + n_sub) * 128, :].rearrange(
                    "(t p) c -> p t c", p=128),
                xs4[:, :n_sub, :])

        # ---- edge: flattened cross-block software pipeline ----
        # Global pair list; stage offsets (conv @i, u+prelu @i-1, z/exp/eg
        # @i-2, agg/den @i-3) run across block boundaries so no engine
        # queue drains at block edges. Block tails are emitted right after
        # the block's last agg/den.
        prefetch_gather(0, splits=6)
        P = K // 2
        pairs = [(b, p) for b in range(BLOCKS) for p in range(P)]
        NPAIR = len(pairs)
        st = {"g8": {}, "agg": {}, "den": {}, "s": {}, "ez": {}, "eg": {}}

        def blk_state(b):
            if b not in st["g8"]:
                g8 = g8p.tile([128, K + 1, HC], fp8, tag="g8")
                st["g8"][b] = g8
                # xr'' for this block -> fp8 slot 0
                lhs = lhsp.tile([128, 128], bf16, tag="lhs")
                nc.sync.dma_start(lhs[:], xlocT[:, b * 128:(b + 1) * 128])
                psr2 = pup.tile([128, 2, HC], f32, tag="uT")
                psr = psr2[:, 0, :]
                nc.tensor.matmul(psr, lhs[:], wr_sb[:], start=True, stop=True)
                if bias_zero:
                    nc.scalar.activation(g8[:, 0, :], psr, AF.Copy)
                else:
                    xrt = densep.tile([128, HC], bf16, tag="xrt")
                    nc.vector.tensor_tensor(out=xrt[:], in0=psr,
                                            in1=brB_sb[:], op=ALU.add)
                    nc.scalar.activation(g8[:, 0, :], xrt[:], AF.Copy)
                agg_t = paggp.tile([128, HC], f32, tag="agg")
                den_t = pdenp.tile([128, 4], f32, tag="den")
                st["agg"][b] = agg_t
                st["den"][b] = den_t
            return st["g8"][b], st["agg"][b], st["den"][b]

        def conv_stage(b, p):
            g8, _, _ = blk_state(b)
            g = blk_g[b]
            for j in (0, 1):
                slot = 2 * p + j + 1
                nc.vector.tensor_scalar(
                    out=g8[:, slot, :], in0=g[:, slot, :],
                    scalar1=1.0, scalar2=None, op0=ALU.mult)
            if p == 15 and b + 1 < BLOCKS:
                prefetch_loads(b + 1)
            if p == 16 and b + 1 < BLOCKS:
                prefetch_gather(b + 1)

        def front(b, p):
            g8, _, _ = blk_state(b)
            ot = blk_loads[b][1]
            uT = pup.tile([128, 2, HC], f32, tag="uT")
            for j in (0, 1):
                k = 2 * p + j
                slot = k + 1
                for h in range(H):
                    lhs_ap = g8[:, 0:slot + 1:slot, h * 128:(h + 1) * 128]
                    rhs_ap = ot[:, k:K + 1:K - k, :]
                    nc.tensor.matmul(uT[:, j, h * 128:(h + 1) * 128],
                                     lhs_ap, rhs_ap, start=True, stop=True,
                                     perf_mode=PM.DoubleRow)
            s_ = sp.tile([128, 2, HC], bf16, tag="s")
            nc.scalar.activation(s_[:], uT[:], AF.Prelu, alpha=alphaP[:])
            st["s"][(b, p)] = s_

        def mid(b, p):
            g = blk_g[b]
            zP = pzp.tile([128, 8], f32, tag="zP")
            s_ = st["s"].pop((b, p))
            for j in (0, 1):
                for h in range(H):
                    nc.tensor.matmul(zP[:, j * 4 + h:j * 4 + h + 1],
                                     s_[:, j, h * 128:(h + 1) * 128],
                                     sgn_sb[:, h:h + 1],
                                     start=True, stop=True)
            ezf = ezp.tile([128, 8], f32, tag="ez")
            nc.scalar.activation(ezf[:], zP[:], AF.Exp)
            ezb = ezp.tile([128, 8], bf16, tag="ezb")
            nc.vector.tensor_scalar(out=ezb[:], in0=ezf[:], scalar1=1.0,
                                    scalar2=None, op0=ALU.mult)
            st["ez"][(b, p)] = ezb
            eg = egp.tile([128, 2, HC], bf16, tag="eg")
            for j in (0, 1):
                k = 2 * p + j
                for h in range(H):
                    nc.gpsimd.tensor_scalar(
                        out=eg[:, j, h * 128:(h + 1) * 128],
                        in0=g[:, k + 1, h * 128:(h + 1) * 128],
                        scalar1=ezf[:, j * 4 + h:j * 4 + h + 1],
                        scalar2=None, op0=ALU.mult)
            st["eg"][(b, p)] = eg

        def back(b, p):
            _, agg, den = blk_state(b)
            ohb = blk_loads[b][2]
            eg = st["eg"].pop((b, p))
            ezb = st["ez"].pop((b, p))
            for j in (0, 1):
                k = 2 * p + j
                nc.tensor.matmul(agg[:], ohb[:, k * BLK:(k + 1) * BLK],
                                 eg[:, j, :],
                                 start=(k == 0), stop=(k == K - 1))
                nc.tensor.matmul(den[:], ohb[:, k * BLK:(k + 1) * BLK],
                                 ezb[:, j * 4:(j + 1) * 4],
                                 start=(k == 0), stop=(k == K - 1))
            if p == P - 1:
                tail(b)

        def tail(b):
            agg = st["agg"].pop(b)
            den = st["den"].pop(b)
            for dd in ("g8",):
                st[dd].pop(b, None)
            blk_g.pop(b, None)
            blk_loads.pop(b, None)
            rden = lgp.tile([128, 4], f32, tag="rden")
            nc.vector.reciprocal(rden[:], den[:])
            tq = lnp.tile([128, HC], f32, tag="tq")
            for h in range(H):
                nc.vector.tensor_scalar(
                    out=tq[:, h * 128:(h + 1) * 128],
                    in0=agg[:, h * 128:(h + 1) * 128],
                    scalar1=rden[:, h:h + 1], scalar2=None, op0=ALU.mult)
            tq2 = lnp.tile([128, HC], f32, tag="tq2")
            nc.vector.tensor_tensor(out=tq2[:], in0=tq[:], in1=invatt_sb[:],
                                    op=ALU.mult)
            hm = outp.tile([128, 128], f32, tag="hm")
            nc.vector.tensor_reduce(
                out=hm[:], in_=tq2[:].rearrange("p (h c) -> p c h", h=H),
                axis=X, op=ALU.add)
            xt = outp.tile([128, 128], f32, tag="xres")
            nc.sync.dma_start(xt[:], xloc[b * 128:(b + 1) * 128, :])
            if bias_zero:
                r2 = outp.tile([128, 128], f32, tag="r2")
                nc.vector.tensor_tensor(out=r2[:], in0=hm[:], in1=xt[:],
                                        op=ALU.add)
            else:
                r1 = outp.tile([128, 128], f32, tag="r1")
                nc.vector.tensor_tensor(out=r1[:], in0=hm[:],
                                        in1=biasB_sb[:], op=ALU.add)
                r2 = outp.tile([128, 128], f32, tag="r2")
                nc.vector.tensor_tensor(out=r2[:], in0=r1[:], in1=xt[:],
                                        op=ALU.add)
            mu = lgp.tile([128, 1], f32, tag="mu")
            nc.vector.tensor_reduce(out=mu[:], in_=r2[:], axis=X, op=ALU.add)
            mun = lgp.tile([128, 1], f32, tag="mun")
            nc.vector.tensor_scalar_mul(mun[:], mu[:], 1.0 / 128)
            xc = outp.tile([128, 128], f32, tag="xc")
            nc.vector.tensor_scalar(out=xc[:], in0=r2[:], scalar1=mun[:],
                                    scalar2=None, op0=ALU.subtract)
            junk = outp.tile([128, 128], f32, tag="junk")
            vs = lgp.tile([128, 1], f32, tag="vs")
            nc.vector.scalar_tensor_tensor(
                out=junk[:], in0=r2[:], scalar=mun[:], in1=xc[:],
                op0=ALU.subtract, op1=ALU.mult, accum_out=vs[:])
            varT = lgp.tile([128, 1], f32, tag="varT")
            nc.vector.tensor_scalar(out=varT[:], in0=vs[:],
                                    scalar1=1.0 / 128, scalar2=LN_EPS,
                                    op0=ALU.mult, op1=ALU.add)
            rvar = lgp.tile([128, 1], f32, tag="rvar")
            nc.vector.reciprocal(rvar[:], varT[:])
            rstd = lgp.tile([128, 1], f32, tag="rstd")
            nc.scalar.activation(rstd[:], rvar[:], AF.Sqrt)
            xn = outp.tile([128, 128], f32, tag="xn")
            nc.vector.tensor_scalar(out=xn[:], in0=xc[:], scalar1=rstd[:],
                                    scalar2=None, op0=ALU.mult)
            if ln_triv:
                xgb = xn
            else:
                xg = outp.tile([128, 128], f32, tag="xg")
                nc.vector.tensor_tensor(out=xg[:], in0=xn[:], in1=lngB_sb[:],
                                        op=ALU.mult)
                xgb = outp.tile([128, 128], f32, tag="xgb")
                nc.vector.tensor_tensor(out=xgb[:], in0=xg[:],
                                        in1=lnbB_sb[:], op=ALU.add)
            xout = outp.tile([128, 128], f32, tag="xout")
            nc.scalar.activation(xout[:], xgb[:], AF.Relu)
            nc.sync.dma_start(xnew[b * 128:(b + 1) * 128, :], xout[:])

        for i in range(NPAIR + 3):
            if i < NPAIR:
                conv_stage(*pairs[i])
            if 1 <= i and i - 1 < NPAIR:
                front(*pairs[i - 1])
            if 2 <= i and i - 2 < NPAIR:
                mid(*pairs[i - 2])
            if 3 <= i and i - 3 < NPAIR:
                back(*pairs[i - 3])

    nc.compile()
    return nc


def kernel(x, edge_index, Wl, bl, Wr, br, att, bias, ln_g, ln_b):
    x = np.asarray(x, np.float32)
    edge_index = np.asarray(edge_index)
    Wl = np.asarray(Wl, np.float32); bl = np.asarray(bl, np.float32)
    Wr = np.asarray(Wr, np.float32); br = np.asarray(br, np.float32)
    att = np.asarray(att, np.float32); bias = np.asarray(bias, np.float32)
    ln_g = np.asarray(ln_g, np.float32); ln_b = np.asarray(ln_b, np.float32)

    K, src_arr, dpos_arr = _prep_edges(edge_index)
    sidx, ohtid, ohflat = _build_ship_arrays(K, src_arr, dpos_arr)

    bias_zero = not (np.any(bias) or np.any(bl) or np.any(br))
    ln_triv = (np.all(ln_g == 1.0) and not np.any(ln_b))
    key = (K, bias_zero, ln_triv)
    if key not in _NC_CACHE:
        _NC_CACHE[key] = _build_nc(K, bias_zero, ln_triv)
    nc = _NC_CACHE[key]

    aatt = np.maximum(np.abs(att), 1e-30)
    sgn = np.sign(att).astype(np.float32)
    sgn[sgn == 0] = 1.0

    LAST_RESULTS.clear()
    cur = x
    for l in range(L):
        a_flat = aatt[l].reshape(HC)
        WlS = (Wl[l] * a_flat[None, :]).astype(BF16)
        WrS = (Wr[l] * a_flat[None, :]).astype(BF16)
        sgnT = np.ascontiguousarray(sgn[l].T).astype(BF16)   # [C, H]

        xpad = np.zeros((N_ROWS, 128), np.float32)
        xpad[:N_NODES] = cur
        xT = np.ascontiguousarray(xpad.T.astype(BF16))
        xloc_full = np.zeros((N_PAD, 128), np.float32)
        xloc_full[:N_NODES] = cur

        common = {
            "xT": xT, "WlS": WlS, "WrS": WrS,
            "blB": _bcast(bl[l] * a_flat), "brB": _bcast(br[l] * a_flat),
            "sgnT": sgnT, "invatt4B": _bcast(0.25 / a_flat),
            "biasB": _bcast(bias[l]), "lngB": _bcast(ln_g[l]),
            "lnbB": _bcast(ln_b[l]),
        }
        in_maps = []
        for c in range(N_CORES):
            xl_c = np.ascontiguousarray(
                xloc_full[c * NODES_PER_CORE:(c + 1) * NODES_PER_CORE])
            in_maps.append({
                **common,
                "xloc": xl_c,
                "xlocT": np.ascontiguousarray(xl_c.T.astype(BF16)),
                "ohtidd": ohtid[c], "ohd": ohflat[c], "sidxd": sidx[c],
            })

        res = run_bass_kernel_spmd(nc, in_maps, core_ids=list(range(N_CORES)))
        LAST_RESULTS.append(res)
        nxt = np.concatenate([res.results[c]["xnew"] for c in range(N_CORES)],
                             axis=0)
        cur = np.ascontiguousarray(nxt[:N_NODES])

    return cur.astype(np.float32)
